# revision 24
# baseline (speedup 1.0000x reference)
"""ConvGRU Trainium2 Bass kernel (fp8 DoubleRow).

Math: ConvGRU cell with 3 gates (z, r, q), each gate = depthwise 3x3 conv
(SAME) followed by pointwise 1x1 conv, weights int8-fake-quantized
per-tensor.

Strategy:
  - Data-parallel over batch: 8 images -> 8 NeuronCores, one image each.
  - The depthwise+pointwise composition is folded into 9 shifted matmuls
    accumulated in PSUM:  p = sum_t (Wp . diag(Wd_t)) @ shift_t(hx).
    Shifts are free-dim AP offsets into zero-padded SBUF images
    (W 128->130, H 64->66).
  - Operands are fp8 e4m3 with perf_mode=DoubleRow: the PE holds 2 fp8
    weights per cell, contracting 256 rows per pass. The 448 channels sit
    in one [128, 4*PP] image as 4 blocks [h | x0 | x1 | x2+zeros]; a
    DoubleRow rhs AP [128, 2, n] pairs adjacent blocks, so a gate-window
    is 18 matmuls instead of 32. Weights are scaled per-gate into fp8
    range (e4m3, max 240); the inverse rides the activation's fused scale. Activation
    images are unscaled (fp8 is a float; values are O(1)).
    Measured end-to-end relative error ~1.1e-2 (gate 2e-2); inputs are
    deterministic (fixed seed in the reference).
  - The q gate reads rh = r*h: phase 1 overwrites the h block in place
    with fp8 rh, lagged one window so the z/r halo reads of row 3w+3
    happen first. Block pads stay zero (host-padded, muls write only
    trimmed interiors), so no device memsets are needed.
  - A bf16 copy of h feeds the rh multiply and the GRU mix exactly.
  - Per-core image is processed in 22 row-windows (3 rows = 388 usable
    cols per matmul, PSUM-bank sized).
"""

import sys

sys.path.insert(0, "/opt/trn_rl_repo")

import ml_dtypes
import numpy as np

HID, INP, C = 128, 320, 448
B, H, W = 8, 64, 128
Wp, Hp = 130, 66
PIX = Hp * Wp  # 8580
PP = PIX + 12  # block pitch; % 16 == 0 for the DoubleRow AP step rule
NPIX = H * W  # 8192
ROWS_PER_WIN = 3
NWIN = 22  # 21 windows x 3 rows + 1 window x 1 row

# taps: center first (it is never clamped, so start=True initializes every
# PSUM column of the accumulation group)
TAPS = sorted(
    [(ky - 1, kx - 1, 3 * ky + kx) for ky in range(3) for kx in range(3)],
    key=lambda t: (t[0] != 0 or t[1] != 0),
)

NBLK = 10  # image blocks in u (see _build_blocks)


def _slot_table():
    """16 DoubleRow slots per gate-window.

    Each slot: (base_block, o_delta, skip_last, wspec); the matmul reads
    u4[:, base:base+2, q0+o_delta : +n] and wspec lists weight entries
    (half, p0, p1, (dy, dx), c0): lhsT[p0:p1, half, :] = m_tap[c0+(p-p0)].
    Blocks: 0=h|rh 1=x0 2=x1<<2Wp 3=x1 4=x1<<2 5=x2zp<<2 6=pA 7=pA<<2Wp
    8=pA 9=pB>>(Wp-1); pA=[x2;x2<<2], pB=[x2;x2<<2Wp]."""
    s = []
    for dy, dx, t in TAPS:  # pair A: (h|rh, x0), taps via offsets
        s.append((0, dy * Wp + dx, dy == 1,
                  [(0, 0, 128, (dy, dx), 0), (1, 0, 128, (dy, dx), 128)]))
    s.append((2, -Wp, False,  # pair C: x1 (+1,0)/(-1,0)
              [(0, 0, 128, (1, 0), 256), (1, 0, 128, (-1, 0), 256)]))
    for dy in (-1, 0, 1):  # pair B: x1 (dy,-1)/(dy,+1)
        s.append((3, dy * Wp - 1, dy == 1,
                  [(0, 0, 128, (dy, -1), 256), (1, 0, 128, (dy, 1), 256)]))
    s.append((4, -2, False,  # pair D: x1 center / x2 center
              [(0, 0, 128, (0, 0), 256), (1, 0, 64, (0, 0), 384)]))
    s.append((6, -Wp - 1, False,  # pair E: x2 corners (quad)
              [(0, 0, 64, (-1, -1), 384), (0, 64, 128, (-1, 1), 384),
               (1, 0, 64, (1, -1), 384), (1, 64, 128, (1, 1), 384)]))
    s.append((8, -1, False,  # pair F: x2 cross (quad)
              [(0, 0, 64, (0, -1), 384), (0, 64, 128, (0, 1), 384),
               (1, 0, 64, (-1, 0), 384), (1, 64, 128, (1, 0), 384)]))
    return s


SLOTS = _slot_table()
NSLOT = len(SLOTS)  # 16


def _shl(a, k):
    out = np.zeros_like(a)
    if k == 0:
        out[:] = a
    elif k > 0:
        out[:, :-k] = a[:, k:]
    else:
        out[:, -k:] = a[:, : a.shape[1] + k]
    return out


def _build_blocks(hpad, x0p, x1p, x2p):
    """10 padded [128, PIX] fp32 image blocks for the slot table."""
    z64 = np.zeros((64, PIX), np.float32)
    x2zp = np.concatenate([x2p, z64], 0)
    pA = np.concatenate([x2p, _shl(x2p, 2)], 0)
    pB = np.concatenate([x2p, _shl(x2p, 2 * Wp)], 0)
    return [hpad, x0p, _shl(x1p, 2 * Wp), x1p, _shl(x1p, 2),
            _shl(x2zp, 2), pA, _shl(pA, 2 * Wp), pA, _shl(pB, -(Wp - 1))]


def _pack_gate_w(lt, G):
    """[128, NSLOT*256] fp32 weight pack from the slot table."""
    p = np.zeros((128, NSLOT, 2, 128), np.float32)
    for si, (_b, _o, _sk, wspec) in enumerate(SLOTS):
        for half, p0, p1, (dy, dx), c0 in wspec:
            t = 3 * (dy + 1) + (dx + 1)
            p[p0:p1, si, half, :] = lt[t][c0 : c0 + (p1 - p0), :] * G
    return p.reshape(128, NSLOT * 256)


_CACHE = {}


def _win_geom(w):
    rows = ROWS_PER_WIN if w < NWIN - 1 else H - ROWS_PER_WIN * (NWIN - 1)
    q0 = (1 + ROWS_PER_WIN * w) * Wp + 1  # first real (non-pad) pixel
    n = rows * Wp - (Wp - W - 1) - 1  # trim leading + trailing pads
    return q0, rows, n


def _build(loop_reps=None):
    """Build the SPMD kernel. loop_reps wraps the whole body in an on-device
    For_i loop (identical code size for any trip count) — used by test.py to
    measure device time as an exec-time slope between two trip counts."""
    import contextlib

    import concourse.bacc as bacc
    import concourse.tile as tile
    from concourse import mybir

    f32 = mybir.dt.float32
    bf16 = mybir.dt.bfloat16
    f8 = mybir.dt.float8e4
    AF = mybir.ActivationFunctionType
    DR = mybir.MatmulPerfMode.DoubleRow

    nc = bacc.Bacc("TRN2", target_bir_lowering=False, debug=False, num_devices=8)

    u_d = nc.dram_tensor("u", [128, NBLK * PP], f8, kind="ExternalInput")
    hb_d = nc.dram_tensor("hb", [128, PIX], bf16, kind="ExternalInput")
    wg_d = {
        g: nc.dram_tensor(f"w{g}", [128, NSLOT * 256], f8, kind="ExternalInput")
        for g in ("z", "r", "q")
    }
    sbt_d = nc.dram_tensor("sbt", [128, 6], f32, kind="ExternalInput")
    out_d = nc.dram_tensor("out", [HID, NPIX], bf16, kind="ExternalOutput")

    GATES = ("z", "r", "q")

    with tile.TileContext(nc) as tc:
        with (
            tc.tile_pool(name="big", bufs=1) as big,
            tc.tile_pool(name="wp", bufs=1) as wpool,
            tc.tile_pool(name="win", bufs=3) as win,
            tc.tile_pool(name="psum", bufs=2, space="PSUM") as psum,
            tc.tile_pool(name="psumw", bufs=1, space="PSUM") as psumw,
        ):
            # PE warm-up: dummy back-to-back matmuls release the HAM clock
            # throttle (K=4/8 -> 8/8) while the first input DMAs run.
            wu = wpool.tile([128, 128], bf16, name="wu")
            nc.vector.memset(wu[:], 0.0)
            pw = psumw.tile([64, 128], f32, name="pw")
            for _ in range(64):
                nc.tensor.matmul(pw[:], wu[:, 0:64], wu[:], start=True, stop=True)

            ctx_loop = (
                tc.For_i(0, loop_reps, 1) if loop_reps else contextlib.nullcontext()
            )
            ctx_loop.__enter__()

            wgt = {g: wpool.tile([128, NSLOT * 256], f8, name=f"w{g}")
                   for g in GATES}
            sbt = wpool.tile([128, 6], f32)
            u = big.tile([128, NBLK * PP], f8)
            hb = big.tile([128, PIX], bf16)
            zb = big.tile([128, PIX], bf16)
            u4 = u.rearrange("p (i q) -> p i q", i=NBLK)

            # head DMAs on the gpsimd (SWDGE) queue in consumption order;
            # the For_i back-edge barrier makes this chain the restart cost.
            splits = [(0, 6), (6, 17), (17, 28), (28, 39), (39, 50), (50, Hp)]

            def usplit(b_, r0, r1):
                # alternate the two free DMA queues: the serial issue chain
                # (~650ns per dma_start) is the bulk of the per-iteration
                # restart cost after the For_i barrier
                sl = slice(b_ * PP + r0 * Wp, b_ * PP + r1 * Wp)
                q_ = nc.gpsimd if b_ % 2 == 0 else nc.sync
                q_.dma_start(out=u[:, sl], in_=u_d[:, sl])

            nc.sync.dma_start(out=wgt["z"][:, :2304], in_=wg_d["z"][:, :2304])
            usplit(0, 0, 6)
            usplit(1, 0, 6)
            nc.sync.dma_start(out=wgt["z"][:, 2304:], in_=wg_d["z"][:, 2304:])
            for b_ in range(2, NBLK):
                usplit(b_, 0, 6)
            nc.gpsimd.dma_start(out=hb[:, 0 : 6 * Wp], in_=hb_d[:, 0 : 6 * Wp])
            nc.gpsimd.dma_start(out=sbt[:], in_=sbt_d[:])
            nc.sync.dma_start(out=wgt["r"][:], in_=wg_d["r"][:])
            for si, (r0, r1) in enumerate(splits[1:], start=1):
                for b_ in range(NBLK):
                    usplit(b_, r0, r1)
                sl = slice(r0 * Wp, r1 * Wp)
                nc.gpsimd.dma_start(out=hb[:, sl], in_=hb_d[:, sl])
                if si == 1:
                    nc.gpsimd.dma_start(out=wgt["q"][:], in_=wg_d["q"][:])

            def issue_gate_mms(pt, g, q0, n, last_win):
                """16 DoubleRow matmuls of one gate for one window."""
                mms = []
                for si, (base, od, skip_last, _w) in enumerate(SLOTS):
                    if last_win and skip_last:
                        continue  # reads only the zero pad row
                    o = q0 + od
                    s, e = max(o, 0), min(o + n, PIX)
                    w3 = wgt[g][:, si * 256 : (si + 1) * 256].rearrange(
                        "p (i m) -> p i m", i=2
                    )
                    mms.append((w3, u4[:, base : base + 2, s:e], s - o, e - s))
                for i, (lhsT, rhs, c0, cn) in enumerate(mms):
                    nc.tensor.matmul(
                        pt[:, c0 : c0 + cn], lhsT, rhs,
                        start=(i == 0), stop=(i == len(mms) - 1),
                        perf_mode=DR,
                    )

            # ---- phase 1: z and r gates; store z; overwrite the h block
            # with fp8 rh = r*h, lagged one window behind the halo reads ----
            rwins = {}
            for w in range(NWIN):
                q0, rows, n = _win_geom(w)
                last = w == NWIN - 1
                pz = psum.tile([128, n], f32, tag="pz", name=f"pz{w}")
                pr = psum.tile([128, n], f32, tag="pr", name=f"pr{w}")
                issue_gate_mms(pz, "z", q0, n, last)
                issue_gate_mms(pr, "r", q0, n, last)
                rwin = win.tile([128, n], bf16, tag="rwin", name=f"rw{w}")
                nc.scalar.activation(
                    rwin[:], pr[:], AF.Sigmoid, bias=sbt[:, 3:4], scale=sbt[:, 2:3]
                )
                nc.scalar.activation(
                    zb[:, q0 : q0 + n], pz[:], AF.Sigmoid,
                    bias=sbt[:, 1:2], scale=sbt[:, 0:1],
                )
                rwins[w] = rwin
                if w > 0:
                    pq0, _, pn = _win_geom(w - 1)
                    nc.vector.tensor_mul(
                        u[:, pq0 : pq0 + pn], rwins[w - 1][:], hb[:, pq0 : pq0 + pn]
                    )
            q0l, _, nl = _win_geom(NWIN - 1)
            nc.vector.tensor_mul(
                u[:, q0l : q0l + nl], rwins[NWIN - 1][:], hb[:, q0l : q0l + nl]
            )

            # ---- phase 2: q gate + GRU mix ----
            out3 = out_d.rearrange("p (r c) -> p r c", c=W)
            for w in range(NWIN):
                q0, rows, n = _win_geom(w)
                last = w == NWIN - 1
                pq = psum.tile([128, n], f32, tag="pq", name=f"pq{w}")
                issue_gate_mms(pq, "q", q0, n, last)
                qwin = win.tile([128, n], bf16, tag="qwin", name=f"qw{w}")
                nc.scalar.activation(
                    qwin[:], pq[:], AF.Tanh, bias=sbt[:, 5:6], scale=sbt[:, 4:5]
                )
                dwin = win.tile([128, n], bf16, tag="dwin", name=f"dw{w}")
                nc.vector.tensor_sub(dwin[:], qwin[:], hb[:, q0 : q0 + n])
                mwin = win.tile([128, n], bf16, tag="mwin", name=f"mw{w}")
                nc.vector.tensor_mul(mwin[:], zb[:, q0 : q0 + n], dwin[:])
                owin = win.tile([128, rows * Wp], bf16, tag="owin", name=f"ow{w}")
                nc.vector.tensor_add(owin[:, :n], hb[:, q0 : q0 + n], mwin[:])
                o3 = owin.rearrange("p (r c) -> p r c", c=Wp)
                y0 = ROWS_PER_WIN * w
                nc.sync.dma_start(
                    out=out3[:, y0 : y0 + rows, :], in_=o3[:, :rows, 0:W]
                )

            ctx_loop.__exit__(None, None, None)

    nc.compile()
    return nc


def _fq_int(w):
    w = np.asarray(w, np.float32)
    scale = (
        np.maximum(np.max(np.abs(w)), np.float32(1e-8)) / np.float32(127.0)
    ).astype(np.float32)
    q = np.clip(np.round(w / scale), -128, 127).astype(np.float32)
    return q, scale


def _prep_gate(wdg, bdg, wpg, bpg):
    """Folded integer lhsT per tap [9, C, HID], plus the activation's
    scale (sd*sp) and bias (sp * Wp@bd + bp)."""
    qd, sd = _fq_int(wdg)  # [C,1,3,3]
    qp, sp = _fq_int(wpg)  # [HID,C,1,1]
    qp2 = qp[:, :, 0, 0]  # [HID, C]
    lhsT = np.empty((9, C, HID), np.float32)
    for ky in range(3):
        for kx in range(3):
            m = qp2 * qd[:, 0, ky, kx][None, :]  # [HID, C]
            lhsT[3 * ky + kx] = m.T
    scale = np.float32(sd) * np.float32(sp)
    bias = (
        np.float32(sp) * (qp2 @ np.asarray(bdg, np.float32))
        + np.asarray(bpg, np.float32)
    ).astype(np.float32)
    return lhsT, scale, bias


def _pad_img(a):
    """[K, 64, 128] -> zero-padded flat [K, PIX] (rows 1..64, cols 1..128)."""
    k = a.shape[0]
    p = np.zeros((k, Hp, Wp), np.float32)
    p[:, 1 : H + 1, 1 : W + 1] = a
    return p.reshape(k, PIX)


GATES_HOST = ("z", "r", "q")


def last_in_maps(inputs):
    bf = ml_dtypes.bfloat16
    f8 = ml_dtypes.float8_e4m3
    h = np.asarray(inputs["h"], np.float32)
    x = np.asarray(inputs["x"], np.float32)

    sbt = np.empty((HID, 6), np.float32)
    wg = {}
    for gi, g in enumerate(GATES_HOST):
        lt, s_, b_ = _prep_gate(
            inputs[f"wd{g}"], inputs[f"bd{g}"], inputs[f"wp{g}"], inputs[f"bp{g}"]
        )
        G = np.float32(240.0) / np.float32(np.max(np.abs(lt)) + 1e-30)
        sbt[:, 2 * gi] = s_ / G
        sbt[:, 2 * gi + 1] = b_
        wg[g] = _pack_gate_w(lt, G).astype(f8)

    in_maps = []
    for i in range(B):
        hpad = _pad_img(h[i])
        blocks = _build_blocks(hpad, _pad_img(x[i, 0:128]),
                               _pad_img(x[i, 128:256]), _pad_img(x[i, 256:320]))
        u = np.zeros((128, NBLK * PP), np.float32)
        for b_, blk in enumerate(blocks):
            u[:, b_ * PP : b_ * PP + PIX] = blk
        in_maps.append(
            {
                "u": u.astype(f8),
                "hb": hpad.astype(bf),
                "wz": wg["z"],
                "wr": wg["r"],
                "wq": wg["q"],
                "sbt": sbt,
            }
        )
    return in_maps


def kernel(**inputs):
    from concourse.bass_utils import run_bass_kernel_spmd

    if "nc" not in _CACHE:
        _CACHE["nc"] = _build()
    nc = _CACHE["nc"]

    in_maps = last_in_maps(inputs)

    res = run_bass_kernel_spmd(nc, in_maps, list(range(B)))
    out = np.stack(
        [
            res.results[i]["out"].astype(np.float32).reshape(HID, H, W)
            for i in range(B)
        ],
        axis=0,
    )
    return out


# revision 25
# speedup vs baseline: 1.0171x; 1.0171x over previous
"""ConvGRU Trainium2 Bass kernel (fp8 DoubleRow).

Math: ConvGRU cell with 3 gates (z, r, q), each gate = depthwise 3x3 conv
(SAME) followed by pointwise 1x1 conv, weights int8-fake-quantized
per-tensor.

Strategy:
  - Data-parallel over batch: 8 images -> 8 NeuronCores, one image each.
  - The depthwise+pointwise composition is folded into 9 shifted matmuls
    accumulated in PSUM:  p = sum_t (Wp . diag(Wd_t)) @ shift_t(hx).
    Shifts are free-dim AP offsets into zero-padded SBUF images
    (W 128->130, H 64->66).
  - Operands are fp8 e4m3 with perf_mode=DoubleRow: the PE holds 2 fp8
    weights per cell, contracting 256 rows per pass. The 448 channels sit
    in one [128, 4*PP] image as 4 blocks [h | x0 | x1 | x2+zeros]; a
    DoubleRow rhs AP [128, 2, n] pairs adjacent blocks, so a gate-window
    is 18 matmuls instead of 32. Weights are scaled per-gate into fp8
    range (e4m3, max 240); the inverse rides the activation's fused scale. Activation
    images are unscaled (fp8 is a float; values are O(1)).
    Measured end-to-end relative error ~1.1e-2 (gate 2e-2); inputs are
    deterministic (fixed seed in the reference).
  - The q gate reads rh = r*h: phase 1 overwrites the h block in place
    with fp8 rh, lagged one window so the z/r halo reads of row 3w+3
    happen first. Block pads stay zero (host-padded, muls write only
    trimmed interiors), so no device memsets are needed.
  - A bf16 copy of h feeds the rh multiply and the GRU mix exactly.
  - Per-core image is processed in 22 row-windows (3 rows = 388 usable
    cols per matmul, PSUM-bank sized).
"""

import sys

sys.path.insert(0, "/opt/trn_rl_repo")

import ml_dtypes
import numpy as np

HID, INP, C = 128, 320, 448
B, H, W = 8, 64, 128
Wp, Hp = 130, 66
PIX = Hp * Wp  # 8580
PP = PIX + 12  # block pitch; % 16 == 0 for the DoubleRow AP step rule
NPIX = H * W  # 8192
ROWS_PER_WIN = 3
NWIN = 22  # 21 windows x 3 rows + 1 window x 1 row

# taps: center first (it is never clamped, so start=True initializes every
# PSUM column of the accumulation group)
TAPS = sorted(
    [(ky - 1, kx - 1, 3 * ky + kx) for ky in range(3) for kx in range(3)],
    key=lambda t: (t[0] != 0 or t[1] != 0),
)

NBLK = 10  # image blocks in u (see _build_blocks)


def _slot_table():
    """16 DoubleRow slots per gate-window.

    Each slot: (base_block, o_delta, skip_last, wspec); the matmul reads
    u4[:, base:base+2, q0+o_delta : +n] and wspec lists weight entries
    (half, p0, p1, (dy, dx), c0): lhsT[p0:p1, half, :] = m_tap[c0+(p-p0)].
    Blocks: 0=h|rh 1=x0 2=x1<<2Wp 3=x1 4=x1<<2 5=x2zp<<2 6=pA 7=pA<<2Wp
    8=pA 9=pB>>(Wp-1); pA=[x2;x2<<2], pB=[x2;x2<<2Wp]."""
    s = []
    for dy, dx, t in TAPS:  # pair A: (h|rh, x0), taps via offsets
        s.append((0, dy * Wp + dx, dy == 1,
                  [(0, 0, 128, (dy, dx), 0), (1, 0, 128, (dy, dx), 128)]))
    s.append((2, -Wp, False,  # pair C: x1 (+1,0)/(-1,0)
              [(0, 0, 128, (1, 0), 256), (1, 0, 128, (-1, 0), 256)]))
    for dy in (-1, 0, 1):  # pair B: x1 (dy,-1)/(dy,+1)
        s.append((3, dy * Wp - 1, dy == 1,
                  [(0, 0, 128, (dy, -1), 256), (1, 0, 128, (dy, 1), 256)]))
    s.append((4, -2, False,  # pair D: x1 center / x2 center
              [(0, 0, 128, (0, 0), 256), (1, 0, 64, (0, 0), 384)]))
    s.append((6, -Wp - 1, False,  # pair E: x2 corners (quad)
              [(0, 0, 64, (-1, -1), 384), (0, 64, 128, (-1, 1), 384),
               (1, 0, 64, (1, -1), 384), (1, 64, 128, (1, 1), 384)]))
    s.append((8, -1, False,  # pair F: x2 cross (quad)
              [(0, 0, 64, (0, -1), 384), (0, 64, 128, (0, 1), 384),
               (1, 0, 64, (-1, 0), 384), (1, 64, 128, (1, 0), 384)]))
    return s


SLOTS = _slot_table()
NSLOT = len(SLOTS)  # 16


def _shl(a, k):
    out = np.zeros_like(a)
    if k == 0:
        out[:] = a
    elif k > 0:
        out[:, :-k] = a[:, k:]
    else:
        out[:, -k:] = a[:, : a.shape[1] + k]
    return out


def _build_blocks(hpad, x0p, x1p, x2p):
    """10 padded [128, PIX] fp32 image blocks for the slot table."""
    z64 = np.zeros((64, PIX), np.float32)
    x2zp = np.concatenate([x2p, z64], 0)
    pA = np.concatenate([x2p, _shl(x2p, 2)], 0)
    pB = np.concatenate([x2p, _shl(x2p, 2 * Wp)], 0)
    return [hpad, x0p, _shl(x1p, 2 * Wp), x1p, _shl(x1p, 2),
            _shl(x2zp, 2), pA, _shl(pA, 2 * Wp), pA, _shl(pB, -(Wp - 1))]


def _pack_gate_w(lt, G):
    """[128, NSLOT*256] fp32 weight pack from the slot table."""
    p = np.zeros((128, NSLOT, 2, 128), np.float32)
    for si, (_b, _o, _sk, wspec) in enumerate(SLOTS):
        for half, p0, p1, (dy, dx), c0 in wspec:
            t = 3 * (dy + 1) + (dx + 1)
            p[p0:p1, si, half, :] = lt[t][c0 : c0 + (p1 - p0), :] * G
    return p.reshape(128, NSLOT * 256)


_CACHE = {}


def _win_geom(w):
    rows = ROWS_PER_WIN if w < NWIN - 1 else H - ROWS_PER_WIN * (NWIN - 1)
    q0 = (1 + ROWS_PER_WIN * w) * Wp + 1  # first real (non-pad) pixel
    n = rows * Wp - (Wp - W - 1) - 1  # trim leading + trailing pads
    return q0, rows, n


def _build(loop_reps=None):
    """Build the SPMD kernel. loop_reps wraps the whole body in an on-device
    For_i loop (identical code size for any trip count) — used by test.py to
    measure device time as an exec-time slope between two trip counts."""
    import contextlib

    import concourse.bacc as bacc
    import concourse.tile as tile
    from concourse import mybir

    f32 = mybir.dt.float32
    bf16 = mybir.dt.bfloat16
    f8 = mybir.dt.float8e4
    AF = mybir.ActivationFunctionType
    DR = mybir.MatmulPerfMode.DoubleRow

    nc = bacc.Bacc("TRN2", target_bir_lowering=False, debug=False, num_devices=8)

    u_d = nc.dram_tensor("u", [128, NBLK * PP], f8, kind="ExternalInput")
    hb_d = nc.dram_tensor("hb", [128, PIX], bf16, kind="ExternalInput")
    wg_d = {
        g: nc.dram_tensor(f"w{g}", [128, NSLOT * 256], f8, kind="ExternalInput")
        for g in ("z", "r", "q")
    }
    sbt_d = nc.dram_tensor("sbt", [128, 6], f32, kind="ExternalInput")
    out_d = nc.dram_tensor("out", [HID, NPIX], bf16, kind="ExternalOutput")

    GATES = ("z", "r", "q")

    with tile.TileContext(nc) as tc:
        with (
            tc.tile_pool(name="big", bufs=1) as big,
            tc.tile_pool(name="wp", bufs=1) as wpool,
            tc.tile_pool(name="win", bufs=3) as win,
            tc.tile_pool(name="psum", bufs=2, space="PSUM") as psum,
            tc.tile_pool(name="psumw", bufs=1, space="PSUM") as psumw,
        ):
            # PE warm-up: dummy back-to-back matmuls release the HAM clock
            # throttle (K=4/8 -> 8/8) while the first input DMAs run.
            wu = wpool.tile([128, 128], bf16, name="wu")
            nc.vector.memset(wu[:], 0.0)
            pw = psumw.tile([64, 128], f32, name="pw")
            for _ in range(64):
                nc.tensor.matmul(pw[:], wu[:, 0:64], wu[:], start=True, stop=True)

            ctx_loop = (
                tc.For_i(0, loop_reps, 1) if loop_reps else contextlib.nullcontext()
            )
            ctx_loop.__enter__()

            wgt = {g: wpool.tile([128, NSLOT * 256], f8, name=f"w{g}")
                   for g in GATES}
            sbt = wpool.tile([128, 6], f32)
            u = big.tile([128, NBLK * PP], f8)
            hb = big.tile([128, PIX], bf16)
            zb = big.tile([128, PIX], bf16)
            u4 = u.rearrange("p (i q) -> p i q", i=NBLK)

            # head DMAs on the gpsimd (SWDGE) queue in consumption order;
            # the For_i back-edge barrier makes this chain the restart cost.
            splits = [(0, 6), (6, 17), (17, 28), (28, 39), (39, 50), (50, Hp)]

            def usplit(b_, r0, r1):
                sl = slice(b_ * PP + r0 * Wp, b_ * PP + r1 * Wp)
                nc.gpsimd.dma_start(out=u[:, sl], in_=u_d[:, sl])

            nc.gpsimd.dma_start(out=wgt["z"][:, :2304], in_=wg_d["z"][:, :2304])
            usplit(0, 0, 6)
            usplit(1, 0, 6)
            nc.gpsimd.dma_start(out=wgt["z"][:, 2304:], in_=wg_d["z"][:, 2304:])
            for b_ in range(2, NBLK):
                usplit(b_, 0, 6)
            nc.gpsimd.dma_start(out=hb[:, 0 : 6 * Wp], in_=hb_d[:, 0 : 6 * Wp])
            nc.gpsimd.dma_start(out=sbt[:], in_=sbt_d[:])
            nc.gpsimd.dma_start(out=wgt["r"][:], in_=wg_d["r"][:])
            for si, (r0, r1) in enumerate(splits[1:], start=1):
                for b_ in range(NBLK):
                    usplit(b_, r0, r1)
                sl = slice(r0 * Wp, r1 * Wp)
                nc.gpsimd.dma_start(out=hb[:, sl], in_=hb_d[:, sl])
                if si == 1:
                    nc.gpsimd.dma_start(out=wgt["q"][:], in_=wg_d["q"][:])

            def issue_gate_mms(pt, g, q0, n, last_win):
                """16 DoubleRow matmuls of one gate for one window."""
                mms = []
                for si, (base, od, skip_last, _w) in enumerate(SLOTS):
                    if last_win and skip_last:
                        continue  # reads only the zero pad row
                    o = q0 + od
                    s, e = max(o, 0), min(o + n, PIX)
                    w3 = wgt[g][:, si * 256 : (si + 1) * 256].rearrange(
                        "p (i m) -> p i m", i=2
                    )
                    mms.append((w3, u4[:, base : base + 2, s:e], s - o, e - s))
                for i, (lhsT, rhs, c0, cn) in enumerate(mms):
                    nc.tensor.matmul(
                        pt[:, c0 : c0 + cn], lhsT, rhs,
                        start=(i == 0), stop=(i == len(mms) - 1),
                        perf_mode=DR,
                    )

            # ---- phase 1: z and r gates; store z; overwrite the h block
            # with fp8 rh = r*h, lagged one window behind the halo reads ----
            rwins = {}
            for w in range(NWIN):
                q0, rows, n = _win_geom(w)
                last = w == NWIN - 1
                pz = psum.tile([128, n], f32, tag="pz", name=f"pz{w}")
                pr = psum.tile([128, n], f32, tag="pr", name=f"pr{w}")
                issue_gate_mms(pz, "z", q0, n, last)
                issue_gate_mms(pr, "r", q0, n, last)
                rwin = win.tile([128, n], bf16, tag="rwin", name=f"rw{w}")
                nc.scalar.activation(
                    rwin[:], pr[:], AF.Sigmoid, bias=sbt[:, 3:4], scale=sbt[:, 2:3]
                )
                nc.scalar.activation(
                    zb[:, q0 : q0 + n], pz[:], AF.Sigmoid,
                    bias=sbt[:, 1:2], scale=sbt[:, 0:1],
                )
                rwins[w] = rwin
                if w > 0:
                    pq0, _, pn = _win_geom(w - 1)
                    nc.vector.tensor_mul(
                        u[:, pq0 : pq0 + pn], rwins[w - 1][:], hb[:, pq0 : pq0 + pn]
                    )
            q0l, _, nl = _win_geom(NWIN - 1)
            nc.vector.tensor_mul(
                u[:, q0l : q0l + nl], rwins[NWIN - 1][:], hb[:, q0l : q0l + nl]
            )

            # ---- phase 2: q gate + GRU mix ----
            out3 = out_d.rearrange("p (r c) -> p r c", c=W)
            for w in range(NWIN):
                q0, rows, n = _win_geom(w)
                last = w == NWIN - 1
                pq = psum.tile([128, n], f32, tag="pq", name=f"pq{w}")
                issue_gate_mms(pq, "q", q0, n, last)
                qwin = win.tile([128, n], bf16, tag="qwin", name=f"qw{w}")
                nc.scalar.activation(
                    qwin[:], pq[:], AF.Tanh, bias=sbt[:, 5:6], scale=sbt[:, 4:5]
                )
                dwin = win.tile([128, n], bf16, tag="dwin", name=f"dw{w}")
                nc.vector.tensor_sub(dwin[:], qwin[:], hb[:, q0 : q0 + n])
                mwin = win.tile([128, n], bf16, tag="mwin", name=f"mw{w}")
                nc.vector.tensor_mul(mwin[:], zb[:, q0 : q0 + n], dwin[:])
                owin = win.tile([128, rows * Wp], bf16, tag="owin", name=f"ow{w}")
                nc.vector.tensor_add(owin[:, :n], hb[:, q0 : q0 + n], mwin[:])
                o3 = owin.rearrange("p (r c) -> p r c", c=Wp)
                y0 = ROWS_PER_WIN * w
                nc.sync.dma_start(
                    out=out3[:, y0 : y0 + rows, :], in_=o3[:, :rows, 0:W]
                )

            ctx_loop.__exit__(None, None, None)

    nc.compile()
    return nc


def _fq_int(w):
    w = np.asarray(w, np.float32)
    scale = (
        np.maximum(np.max(np.abs(w)), np.float32(1e-8)) / np.float32(127.0)
    ).astype(np.float32)
    q = np.clip(np.round(w / scale), -128, 127).astype(np.float32)
    return q, scale


def _prep_gate(wdg, bdg, wpg, bpg):
    """Folded integer lhsT per tap [9, C, HID], plus the activation's
    scale (sd*sp) and bias (sp * Wp@bd + bp)."""
    qd, sd = _fq_int(wdg)  # [C,1,3,3]
    qp, sp = _fq_int(wpg)  # [HID,C,1,1]
    qp2 = qp[:, :, 0, 0]  # [HID, C]
    lhsT = np.empty((9, C, HID), np.float32)
    for ky in range(3):
        for kx in range(3):
            m = qp2 * qd[:, 0, ky, kx][None, :]  # [HID, C]
            lhsT[3 * ky + kx] = m.T
    scale = np.float32(sd) * np.float32(sp)
    bias = (
        np.float32(sp) * (qp2 @ np.asarray(bdg, np.float32))
        + np.asarray(bpg, np.float32)
    ).astype(np.float32)
    return lhsT, scale, bias


def _pad_img(a):
    """[K, 64, 128] -> zero-padded flat [K, PIX] (rows 1..64, cols 1..128)."""
    k = a.shape[0]
    p = np.zeros((k, Hp, Wp), np.float32)
    p[:, 1 : H + 1, 1 : W + 1] = a
    return p.reshape(k, PIX)


GATES_HOST = ("z", "r", "q")


def last_in_maps(inputs):
    bf = ml_dtypes.bfloat16
    f8 = ml_dtypes.float8_e4m3
    h = np.asarray(inputs["h"], np.float32)
    x = np.asarray(inputs["x"], np.float32)

    sbt = np.empty((HID, 6), np.float32)
    wg = {}
    for gi, g in enumerate(GATES_HOST):
        lt, s_, b_ = _prep_gate(
            inputs[f"wd{g}"], inputs[f"bd{g}"], inputs[f"wp{g}"], inputs[f"bp{g}"]
        )
        G = np.float32(240.0) / np.float32(np.max(np.abs(lt)) + 1e-30)
        sbt[:, 2 * gi] = s_ / G
        sbt[:, 2 * gi + 1] = b_
        wg[g] = _pack_gate_w(lt, G).astype(f8)

    in_maps = []
    for i in range(B):
        hpad = _pad_img(h[i])
        blocks = _build_blocks(hpad, _pad_img(x[i, 0:128]),
                               _pad_img(x[i, 128:256]), _pad_img(x[i, 256:320]))
        u = np.zeros((128, NBLK * PP), np.float32)
        for b_, blk in enumerate(blocks):
            u[:, b_ * PP : b_ * PP + PIX] = blk
        in_maps.append(
            {
                "u": u.astype(f8),
                "hb": hpad.astype(bf),
                "wz": wg["z"],
                "wr": wg["r"],
                "wq": wg["q"],
                "sbt": sbt,
            }
        )
    return in_maps


def kernel(**inputs):
    from concourse.bass_utils import run_bass_kernel_spmd

    if "nc" not in _CACHE:
        _CACHE["nc"] = _build()
    nc = _CACHE["nc"]

    in_maps = last_in_maps(inputs)

    res = run_bass_kernel_spmd(nc, in_maps, list(range(B)))
    out = np.stack(
        [
            res.results[i]["out"].astype(np.float32).reshape(HID, H, W)
            for i in range(B)
        ],
        axis=0,
    )
    return out


# revision 26
# speedup vs baseline: 1.0378x; 1.0203x over previous
"""ConvGRU Trainium2 Bass kernel (fp8 DoubleRow).

Math: ConvGRU cell with 3 gates (z, r, q), each gate = depthwise 3x3 conv
(SAME) followed by pointwise 1x1 conv, weights int8-fake-quantized
per-tensor.

Strategy:
  - Data-parallel over batch: 8 images -> 8 NeuronCores, one image each.
  - The depthwise+pointwise composition is folded into 9 shifted matmuls
    accumulated in PSUM:  p = sum_t (Wp . diag(Wd_t)) @ shift_t(hx).
    Shifts are free-dim AP offsets into zero-padded SBUF images
    (W 128->130, H 64->66).
  - Operands are fp8 e4m3 with perf_mode=DoubleRow: the PE holds 2 fp8
    weights per cell, contracting 256 rows per pass. The 448 channels sit
    in one [128, 4*PP] image as 4 blocks [h | x0 | x1 | x2+zeros]; a
    DoubleRow rhs AP [128, 2, n] pairs adjacent blocks, so a gate-window
    is 18 matmuls instead of 32. Weights are scaled per-gate into fp8
    range (e4m3, max 240); the inverse rides the activation's fused scale. Activation
    images are unscaled (fp8 is a float; values are O(1)).
    Measured end-to-end relative error ~1.1e-2 (gate 2e-2); inputs are
    deterministic (fixed seed in the reference).
  - The q gate reads rh = r*h: phase 1 overwrites the h block in place
    with fp8 rh, lagged one window so the z/r halo reads of row 3w+3
    happen first. Block pads stay zero (host-padded, muls write only
    trimmed interiors), so no device memsets are needed.
  - A bf16 copy of h feeds the rh multiply and the GRU mix exactly.
  - Per-core image is processed in 22 row-windows (3 rows = 388 usable
    cols per matmul, PSUM-bank sized).
"""

import sys

sys.path.insert(0, "/opt/trn_rl_repo")

import ml_dtypes
import numpy as np

HID, INP, C = 128, 320, 448
B, H, W = 8, 64, 128
Wp, Hp = 130, 66
PIX = Hp * Wp  # 8580
PP = PIX + 12  # block pitch; % 16 == 0 for the DoubleRow AP step rule
NPIX = H * W  # 8192
ROWS_PER_WIN = 3
NWIN = 22  # 21 windows x 3 rows + 1 window x 1 row

# taps: center first (it is never clamped, so start=True initializes every
# PSUM column of the accumulation group)
TAPS = sorted(
    [(ky - 1, kx - 1, 3 * ky + kx) for ky in range(3) for kx in range(3)],
    key=lambda t: (t[0] != 0 or t[1] != 0),
)

NBLK = 10  # image blocks in u (see _build_blocks)


def _slot_table():
    """16 DoubleRow slots per gate-window.

    Each slot: (base_block, o_delta, skip_last, wspec); the matmul reads
    u4[:, base:base+2, q0+o_delta : +n] and wspec lists weight entries
    (half, p0, p1, (dy, dx), c0): lhsT[p0:p1, half, :] = m_tap[c0+(p-p0)].
    Blocks: 0=h|rh 1=x0 2=x1<<2Wp 3=x1 4=x1<<2 5=x2zp<<2 6=pA 7=pA<<2Wp
    8=pA 9=pB>>(Wp-1); pA=[x2;x2<<2], pB=[x2;x2<<2Wp]."""
    s = []
    for dy, dx, t in TAPS:  # pair A: (h|rh, x0), taps via offsets
        s.append((0, dy * Wp + dx, dy == 1,
                  [(0, 0, 128, (dy, dx), 0), (1, 0, 128, (dy, dx), 128)]))
    s.append((2, -Wp, False,  # pair C: x1 (+1,0)/(-1,0)
              [(0, 0, 128, (1, 0), 256), (1, 0, 128, (-1, 0), 256)]))
    for dy in (-1, 0, 1):  # pair B: x1 (dy,-1)/(dy,+1)
        s.append((3, dy * Wp - 1, dy == 1,
                  [(0, 0, 128, (dy, -1), 256), (1, 0, 128, (dy, 1), 256)]))
    s.append((4, -2, False,  # pair D: x1 center / x2 center
              [(0, 0, 128, (0, 0), 256), (1, 0, 64, (0, 0), 384)]))
    s.append((6, -Wp - 1, False,  # pair E: x2 corners (quad)
              [(0, 0, 64, (-1, -1), 384), (0, 64, 128, (-1, 1), 384),
               (1, 0, 64, (1, -1), 384), (1, 64, 128, (1, 1), 384)]))
    s.append((8, -1, False,  # pair F: x2 cross (quad)
              [(0, 0, 64, (0, -1), 384), (0, 64, 128, (0, 1), 384),
               (1, 0, 64, (-1, 0), 384), (1, 64, 128, (1, 0), 384)]))
    return s


SLOTS = _slot_table()
NSLOT = len(SLOTS)  # 16


def _shl(a, k):
    out = np.zeros_like(a)
    if k == 0:
        out[:] = a
    elif k > 0:
        out[:, :-k] = a[:, k:]
    else:
        out[:, -k:] = a[:, : a.shape[1] + k]
    return out


def _build_blocks(hpad, x0p, x1p, x2p):
    """10 padded [128, PIX] fp32 image blocks for the slot table."""
    z64 = np.zeros((64, PIX), np.float32)
    x2zp = np.concatenate([x2p, z64], 0)
    pA = np.concatenate([x2p, _shl(x2p, 2)], 0)
    pB = np.concatenate([x2p, _shl(x2p, 2 * Wp)], 0)
    return [hpad, x0p, _shl(x1p, 2 * Wp), x1p, _shl(x1p, 2),
            _shl(x2zp, 2), pA, _shl(pA, 2 * Wp), pA, _shl(pB, -(Wp - 1))]


def _pack_gate_w(lt, G):
    """[128, NSLOT*256] fp32 weight pack from the slot table."""
    p = np.zeros((128, NSLOT, 2, 128), np.float32)
    for si, (_b, _o, _sk, wspec) in enumerate(SLOTS):
        for half, p0, p1, (dy, dx), c0 in wspec:
            t = 3 * (dy + 1) + (dx + 1)
            p[p0:p1, si, half, :] = lt[t][c0 : c0 + (p1 - p0), :] * G
    return p.reshape(128, NSLOT * 256)


_CACHE = {}


def _win_geom(w):
    rows = ROWS_PER_WIN if w < NWIN - 1 else H - ROWS_PER_WIN * (NWIN - 1)
    q0 = (1 + ROWS_PER_WIN * w) * Wp + 1  # first real (non-pad) pixel
    n = rows * Wp - (Wp - W - 1) - 1  # trim leading + trailing pads
    return q0, rows, n


def _build(loop_reps=None):
    """Build the SPMD kernel. loop_reps wraps the whole body in an on-device
    For_i loop (identical code size for any trip count) — used by test.py to
    measure device time as an exec-time slope between two trip counts."""
    import contextlib

    import concourse.bacc as bacc
    import concourse.tile as tile
    from concourse import mybir

    f32 = mybir.dt.float32
    bf16 = mybir.dt.bfloat16
    f8 = mybir.dt.float8e4
    AF = mybir.ActivationFunctionType
    DR = mybir.MatmulPerfMode.DoubleRow

    nc = bacc.Bacc("TRN2", target_bir_lowering=False, debug=False, num_devices=8)

    u_d = nc.dram_tensor("u", [128, NBLK * PP], f8, kind="ExternalInput")
    hb_d = nc.dram_tensor("hb", [128, PIX], bf16, kind="ExternalInput")
    wg_d = {
        g: nc.dram_tensor(f"w{g}", [128, NSLOT * 256], f8, kind="ExternalInput")
        for g in ("z", "r", "q")
    }
    sbt_d = nc.dram_tensor("sbt", [128, 6], f32, kind="ExternalInput")
    out_d = nc.dram_tensor("out", [HID, NPIX], bf16, kind="ExternalOutput")

    GATES = ("z", "r", "q")

    with tile.TileContext(nc) as tc:
        with (
            tc.tile_pool(name="big", bufs=1) as big,
            tc.tile_pool(name="wp", bufs=1) as wpool,
            tc.tile_pool(name="win", bufs=3) as win,
            tc.tile_pool(name="psum", bufs=2, space="PSUM") as psum,
            tc.tile_pool(name="psumw", bufs=1, space="PSUM") as psumw,
        ):
            # PE warm-up: dummy back-to-back matmuls release the HAM clock
            # throttle (K=4/8 -> 8/8) while the first input DMAs run.
            wu = wpool.tile([128, 128], bf16, name="wu")
            nc.vector.memset(wu[:], 0.0)
            pw = psumw.tile([64, 128], f32, name="pw")
            for _ in range(64):
                nc.tensor.matmul(pw[:], wu[:, 0:64], wu[:], start=True, stop=True)

            ctx_loop = (
                tc.For_i(0, loop_reps, 1) if loop_reps else contextlib.nullcontext()
            )
            ctx_loop.__enter__()

            wgt = {g: wpool.tile([128, NSLOT * 256], f8, name=f"w{g}")
                   for g in GATES}
            sbt = wpool.tile([128, 6], f32)
            u = big.tile([128, NBLK * PP], f8)
            hb = big.tile([128, PIX], bf16)
            zb = big.tile([128, PIX], bf16)
            u4 = u.rearrange("p (i q) -> p i q", i=NBLK)

            # head DMAs on the gpsimd (SWDGE) queue in consumption order;
            # the For_i back-edge barrier makes this chain the restart cost.
            splits = [(0, 6), (6, 17), (17, 28), (28, 39), (39, 50), (50, Hp)]

            def usplit(b_, r0, r1):
                sl = slice(b_ * PP + r0 * Wp, b_ * PP + r1 * Wp)
                nc.gpsimd.dma_start(out=u[:, sl], in_=u_d[:, sl])

            # the 8 shifted x-blocks load as ONE 3D DMA per split (8x fewer
            # sequencer issues; the serial issue chain after the For_i
            # barrier is the per-iteration restart cost)
            u4d = u_d.rearrange("p (i q) -> p i q", i=NBLK)

            def xsplit(r0, r1):
                sl = slice(r0 * Wp, r1 * Wp)
                nc.gpsimd.dma_start(out=u4[:, 2:, sl], in_=u4d[:, 2:, sl])

            nc.gpsimd.dma_start(out=wgt["z"][:, :2304], in_=wg_d["z"][:, :2304])
            usplit(0, 0, 6)
            usplit(1, 0, 6)
            nc.gpsimd.dma_start(out=wgt["z"][:, 2304:], in_=wg_d["z"][:, 2304:])
            xsplit(0, 6)
            nc.gpsimd.dma_start(out=hb[:, 0 : 6 * Wp], in_=hb_d[:, 0 : 6 * Wp])
            nc.gpsimd.dma_start(out=sbt[:], in_=sbt_d[:])
            nc.gpsimd.dma_start(out=wgt["r"][:], in_=wg_d["r"][:])
            for si, (r0, r1) in enumerate(splits[1:], start=1):
                usplit(0, r0, r1)
                usplit(1, r0, r1)
                xsplit(r0, r1)
                sl = slice(r0 * Wp, r1 * Wp)
                nc.gpsimd.dma_start(out=hb[:, sl], in_=hb_d[:, sl])
                if si == 1:
                    nc.gpsimd.dma_start(out=wgt["q"][:], in_=wg_d["q"][:])

            def issue_gate_mms(pt, g, q0, n, last_win):
                """16 DoubleRow matmuls of one gate for one window."""
                mms = []
                for si, (base, od, skip_last, _w) in enumerate(SLOTS):
                    if last_win and skip_last:
                        continue  # reads only the zero pad row
                    o = q0 + od
                    s, e = max(o, 0), min(o + n, PIX)
                    w3 = wgt[g][:, si * 256 : (si + 1) * 256].rearrange(
                        "p (i m) -> p i m", i=2
                    )
                    mms.append((w3, u4[:, base : base + 2, s:e], s - o, e - s))
                for i, (lhsT, rhs, c0, cn) in enumerate(mms):
                    nc.tensor.matmul(
                        pt[:, c0 : c0 + cn], lhsT, rhs,
                        start=(i == 0), stop=(i == len(mms) - 1),
                        perf_mode=DR,
                    )

            # ---- phase 1: z and r gates; store z; overwrite the h block
            # with fp8 rh = r*h, lagged one window behind the halo reads ----
            rwins = {}
            for w in range(NWIN):
                q0, rows, n = _win_geom(w)
                last = w == NWIN - 1
                pz = psum.tile([128, n], f32, tag="pz", name=f"pz{w}")
                pr = psum.tile([128, n], f32, tag="pr", name=f"pr{w}")
                issue_gate_mms(pz, "z", q0, n, last)
                issue_gate_mms(pr, "r", q0, n, last)
                rwin = win.tile([128, n], bf16, tag="rwin", name=f"rw{w}")
                nc.scalar.activation(
                    rwin[:], pr[:], AF.Sigmoid, bias=sbt[:, 3:4], scale=sbt[:, 2:3]
                )
                nc.scalar.activation(
                    zb[:, q0 : q0 + n], pz[:], AF.Sigmoid,
                    bias=sbt[:, 1:2], scale=sbt[:, 0:1],
                )
                rwins[w] = rwin
                if w > 0:
                    pq0, _, pn = _win_geom(w - 1)
                    nc.vector.tensor_mul(
                        u[:, pq0 : pq0 + pn], rwins[w - 1][:], hb[:, pq0 : pq0 + pn]
                    )
            q0l, _, nl = _win_geom(NWIN - 1)
            nc.vector.tensor_mul(
                u[:, q0l : q0l + nl], rwins[NWIN - 1][:], hb[:, q0l : q0l + nl]
            )

            # ---- phase 2: q gate + GRU mix ----
            out3 = out_d.rearrange("p (r c) -> p r c", c=W)
            for w in range(NWIN):
                q0, rows, n = _win_geom(w)
                last = w == NWIN - 1
                pq = psum.tile([128, n], f32, tag="pq", name=f"pq{w}")
                issue_gate_mms(pq, "q", q0, n, last)
                qwin = win.tile([128, n], bf16, tag="qwin", name=f"qw{w}")
                nc.scalar.activation(
                    qwin[:], pq[:], AF.Tanh, bias=sbt[:, 5:6], scale=sbt[:, 4:5]
                )
                dwin = win.tile([128, n], bf16, tag="dwin", name=f"dw{w}")
                nc.vector.tensor_sub(dwin[:], qwin[:], hb[:, q0 : q0 + n])
                mwin = win.tile([128, n], bf16, tag="mwin", name=f"mw{w}")
                nc.vector.tensor_mul(mwin[:], zb[:, q0 : q0 + n], dwin[:])
                owin = win.tile([128, rows * Wp], bf16, tag="owin", name=f"ow{w}")
                nc.vector.tensor_add(owin[:, :n], hb[:, q0 : q0 + n], mwin[:])
                o3 = owin.rearrange("p (r c) -> p r c", c=Wp)
                y0 = ROWS_PER_WIN * w
                nc.sync.dma_start(
                    out=out3[:, y0 : y0 + rows, :], in_=o3[:, :rows, 0:W]
                )

            ctx_loop.__exit__(None, None, None)

    nc.compile()
    return nc


def _fq_int(w):
    w = np.asarray(w, np.float32)
    scale = (
        np.maximum(np.max(np.abs(w)), np.float32(1e-8)) / np.float32(127.0)
    ).astype(np.float32)
    q = np.clip(np.round(w / scale), -128, 127).astype(np.float32)
    return q, scale


def _prep_gate(wdg, bdg, wpg, bpg):
    """Folded integer lhsT per tap [9, C, HID], plus the activation's
    scale (sd*sp) and bias (sp * Wp@bd + bp)."""
    qd, sd = _fq_int(wdg)  # [C,1,3,3]
    qp, sp = _fq_int(wpg)  # [HID,C,1,1]
    qp2 = qp[:, :, 0, 0]  # [HID, C]
    lhsT = np.empty((9, C, HID), np.float32)
    for ky in range(3):
        for kx in range(3):
            m = qp2 * qd[:, 0, ky, kx][None, :]  # [HID, C]
            lhsT[3 * ky + kx] = m.T
    scale = np.float32(sd) * np.float32(sp)
    bias = (
        np.float32(sp) * (qp2 @ np.asarray(bdg, np.float32))
        + np.asarray(bpg, np.float32)
    ).astype(np.float32)
    return lhsT, scale, bias


def _pad_img(a):
    """[K, 64, 128] -> zero-padded flat [K, PIX] (rows 1..64, cols 1..128)."""
    k = a.shape[0]
    p = np.zeros((k, Hp, Wp), np.float32)
    p[:, 1 : H + 1, 1 : W + 1] = a
    return p.reshape(k, PIX)


GATES_HOST = ("z", "r", "q")


def last_in_maps(inputs):
    bf = ml_dtypes.bfloat16
    f8 = ml_dtypes.float8_e4m3
    h = np.asarray(inputs["h"], np.float32)
    x = np.asarray(inputs["x"], np.float32)

    sbt = np.empty((HID, 6), np.float32)
    wg = {}
    for gi, g in enumerate(GATES_HOST):
        lt, s_, b_ = _prep_gate(
            inputs[f"wd{g}"], inputs[f"bd{g}"], inputs[f"wp{g}"], inputs[f"bp{g}"]
        )
        G = np.float32(240.0) / np.float32(np.max(np.abs(lt)) + 1e-30)
        sbt[:, 2 * gi] = s_ / G
        sbt[:, 2 * gi + 1] = b_
        wg[g] = _pack_gate_w(lt, G).astype(f8)

    in_maps = []
    for i in range(B):
        hpad = _pad_img(h[i])
        blocks = _build_blocks(hpad, _pad_img(x[i, 0:128]),
                               _pad_img(x[i, 128:256]), _pad_img(x[i, 256:320]))
        u = np.zeros((128, NBLK * PP), np.float32)
        for b_, blk in enumerate(blocks):
            u[:, b_ * PP : b_ * PP + PIX] = blk
        in_maps.append(
            {
                "u": u.astype(f8),
                "hb": hpad.astype(bf),
                "wz": wg["z"],
                "wr": wg["r"],
                "wq": wg["q"],
                "sbt": sbt,
            }
        )
    return in_maps


def kernel(**inputs):
    from concourse.bass_utils import run_bass_kernel_spmd

    if "nc" not in _CACHE:
        _CACHE["nc"] = _build()
    nc = _CACHE["nc"]

    in_maps = last_in_maps(inputs)

    res = run_bass_kernel_spmd(nc, in_maps, list(range(B)))
    out = np.stack(
        [
            res.results[i]["out"].astype(np.float32).reshape(HID, H, W)
            for i in range(B)
        ],
        axis=0,
    )
    return out
